# revision 2
# baseline (speedup 1.0000x reference)
"""GAT (graph attention) layer on 8 TRN2 NeuronCores via Bass/Tile.

Design v2: degree-sorted dst windows, dst-on-partition layout.

Host (index manipulation only):
  - Append self-loops; compute in-degree per dst node.
  - Sort nodes by degree desc; position q -> global window q//128,
    partition q%128.  Global window w -> core w%8, local window w//8,
    so the 8 cores' i-th windows have near-identical max degree and the
    SPMD graph (shared across cores) pads to the max cap with ~0 waste.
  - Within a window, dst d (partition p) owns row p; its edges occupy
    columns.  Column capacity = window max degree (near-uniform thanks
    to the sort).  Pad slots point at an all-zero table row; the known
    pad count per dst is subtracted from the softmax denominator.
  - Table rows are addressed by int16 gather indices (max 32768 rows),
    so each edge is routed to one of two overlapping row ranges
    A=[0,32768) / B=[17408,50176) of ONE table; per-dst edge counts are
    balanced between A and B (free choice for srcs in the overlap).

Kernel per core:
  Phase 1: h|a_s|a_d node table [50176, 264of384] bf16 built with one
    matmul per 128 nodes (extended weights [W | as_vec | ad_vec]); a_d
    for own dst windows kept in SBUF.
  Phase 2: per group of windows, one batched dma_gather per table range
    pulls [h|a_s] rows for all edge slots (dst=partition, edge=col).
    Scores z = a_s + a_d(broadcast per partition); ex = exp(leaky(z))
    written into the gathered tile's spare cols; h *= ex in place; PSUM
    accumulation over edge columns via identity-lhsT matmuls gives
    numerator and denominator in one [128, 264] accumulator.
    out = num/(den - padcnt*ex_pad) + bias.
No collectives; host concatenates/unsorts the 8 dst shards.
"""
import sys

sys.path.insert(0, "/opt/trn_rl_repo")

import os
import numpy as np
import ml_dtypes

N_NODES = 50000
N_EDGES = 1600000
IN_DIM = 128
OUT_DIM = 64
HEADS = 4
HF = HEADS * OUT_DIM  # 256
NEG_SLOPE = 0.2
N_CORES = 8

OFF = 64                 # node n -> table row n + OFF
ROWS = 50176             # 392 * 128 table rows
GW = ROWS // 128         # 392 global windows
LW = GW // N_CORES       # 49 local windows per core
A_HI = 32768             # range A rows [0, A_HI)
B_LO = ROWS - 32768      # range B rows [B_LO, ROWS) = 17408
TBL_ROW = 384            # bf16 elems per table row (768B stride)
TBL_USED = 264           # cols written: 256 h + 4 a_s + 4 a_d
XCH = 1792               # phase-1 chunk cols (14 tiles of 128)
N_XCH = ROWS // XCH      # 28
GCOLS_MAX = int(os.environ.get("GCOLS", "96"))  # gather batch cols
GATHER_PREP = os.environ.get("GPREP", "0") == "1"
SINGLE_PACKET = os.environ.get("GSP", "0") == "1"


def _prep_edges(src, dst):
    """Degree-sort + A/B balance + slot layout. Index manipulation only.

    Self-loop edges are EXCLUDED by the caller; they are handled by the
    kernel's direct per-window path.  Balancing uses a per-window target
    (half the window max degree) so capA+capB stays within ~2 of the
    window max degree."""
    E = src.size
    deg = np.bincount(dst, minlength=N_NODES)

    row = src + OFF
    canA = row < A_HI
    canB = row >= B_LO
    catA = canA & ~canB
    catm = canA & canB
    af = np.bincount(dst[catA], minlength=N_NODES)
    mf = np.bincount(dst[catm], minlength=N_NODES)
    bf = deg - af - mf

    # sort nodes by (deg desc, af-bf) so each local group of 1024 nodes is
    # homogeneous in both degree and forced-A/forced-B counts; the group
    # cap bound is max(wmax, maxaf+maxbf)
    order = np.lexsort((af - bf, -deg))
    pos = np.empty(N_NODES, np.int64)
    pos[order] = np.arange(N_NODES)

    p_e = pos[dst]
    gwin = p_e >> 7
    part = p_e & 127
    core = gwin % N_CORES
    lwin = gwin // N_CORES

    z0 = np.zeros(ROWS, np.int64)
    deg_s = z0.copy()
    deg_s[pos] = deg
    af_s = z0.copy()
    af_s[pos] = af
    bf_s = z0.copy()
    bf_s[pos] = bf
    lwmax = deg_s.reshape(LW, 1024).max(1)
    lmaxaf = af_s.reshape(LW, 1024).max(1)
    lmaxbf = bf_s.reshape(LW, 1024).max(1)
    # per-local-group split target achieving max(wmax, maxaf+maxbf)
    t1_l = np.maximum(lmaxaf, lwmax - lmaxbf)
    t1_n = t1_l[pos // 1024]
    nA = np.minimum(np.maximum(t1_n, af), af + mf)
    nA = np.minimum(nA, deg)
    nB = deg - nA

    side = np.where(catA, 0, 1).astype(np.int64)
    em = np.where(catm)[0]
    dm = dst[em]
    om = np.argsort(dm, kind="stable")
    offm = np.zeros(N_NODES, np.int64)
    np.cumsum(mf[:-1], out=offm[1:])
    rank_m = np.arange(em.size) - offm[dm[om]]
    side[em[om]] = np.where(rank_m < (nA - af)[dm[om]], 0, 1)

    key = dst * 2 + side
    ok = np.argsort(key, kind="stable")
    cnt2 = np.bincount(key, minlength=2 * N_NODES)
    off2 = np.zeros(2 * N_NODES, np.int64)
    np.cumsum(cnt2[:-1], out=off2[1:])
    col = np.empty(E, np.int64)
    col[ok] = np.arange(E) - off2[key[ok]]

    nA_s = np.zeros(ROWS, np.int64)
    nB_s = np.zeros(ROWS, np.int64)
    nA_s[pos] = nA
    nB_s[pos] = nB
    lcapA = nA_s.reshape(LW, 1024).max(1)
    lcapB = nB_s.reshape(LW, 1024).max(1)

    # pad count per (gwin, part)
    padc_g = (
        (lcapA + lcapB)[np.arange(GW) // N_CORES][:, None]
        - (nA_s + nB_s).reshape(GW, 128)
    ).astype(np.float32)

    # group consecutive local windows, total cols <= GCOLS_MAX
    groups = []  # (w0, w1, Ac, Bc, goff_slots)
    w0 = 0
    goff = 0
    while w0 < LW:
        w1 = w0
        Ac = Bc = 0
        while w1 < LW:
            a2, b2 = Ac + int(lcapA[w1]), Bc + int(lcapB[w1])
            if w1 > w0 and a2 + b2 > GCOLS_MAX:
                break
            Ac, Bc = a2, b2
            w1 += 1
        groups.append((w0, w1, Ac, Bc, goff))
        goff += 128 * (Ac + Bc)
        w0 = w1
    TOT = goff

    # per-window col offsets inside the group
    aoff = np.zeros(LW, np.int64)
    boff = np.zeros(LW, np.int64)
    gidx = np.zeros(LW, np.int64)  # window -> group id
    for gi, (w0g, w1g, Ac, Bc, go) in enumerate(groups):
        a = 0
        b = 0
        for w in range(w0g, w1g):
            gidx[w] = gi
            aoff[w] = a
            boff[w] = Ac + b
            a += int(lcapA[w])
            b += int(lcapB[w])

    # slot index arrays
    goff_arr = np.array([g[4] for g in groups], np.int64)
    e_lw = lwin
    base = goff_arr[gidx[e_lw]]
    colg = np.where(side == 0, aoff[e_lw] + col, boff[e_lw] + col)
    slot = base + colg * 128 + part
    val = np.where(side == 0, row, row - B_LO).astype(np.int16)

    # prefill: A regions -> 0 (sentinel row 0), B regions -> 32767
    flat0 = np.empty(TOT, np.int16)
    for (w0g, w1g, Ac, Bc, go) in groups:
        flat0[go : go + 128 * Ac] = 0
        flat0[go + 128 * Ac : go + 128 * (Ac + Bc)] = np.int16(ROWS - 1 - B_LO)
    flat = np.tile(flat0, (N_CORES, 1))
    flat[core, slot] = val

    # wrap to [16, TOT/16] then tile to [128, TOT/16] per core
    sd16 = [
        np.tile(f.reshape(TOT // 16, 16).T.copy(), (8, 1)) for f in flat
    ]
    padcnt = [
        padc_g[c::N_CORES, :].T.copy() for c in range(N_CORES)
    ]  # [128, LW]
    return order, pos, sd16, padcnt, lcapA, lcapB, groups, aoff, boff, TOT


def _build_graph(lcapA, lcapB, groups, aoff, boff, TOT):
    from concourse import bacc, bass, mybir, tile
    from concourse import library_config

    f32 = mybir.dt.float32
    bf16 = mybir.dt.bfloat16
    i16 = mybir.dt.int16

    nc = bacc.Bacc("TRN2", target_bir_lowering=False, debug=False)

    xT = nc.declare_dram_parameter("xT", [IN_DIM, ROWS], bf16, isOutput=False)
    xTo = nc.declare_dram_parameter("xTo", [IN_DIM, LW * 128], bf16, isOutput=False)
    wp_p = nc.declare_dram_parameter("wp", [IN_DIM, HF], f32, isOutput=False)
    wn_p = nc.declare_dram_parameter("wn", [IN_DIM, HF], f32, isOutput=False)
    att_p = nc.declare_dram_parameter("att", [128, 2 * HF], f32, isOutput=False)
    bias_p = nc.declare_dram_parameter("bias", [128, HF], f32, isOutput=False)
    ident_p = nc.declare_dram_parameter("ident", [128, 128], f32, isOutput=False)
    sd16_p = nc.declare_dram_parameter("sd16", [128, TOT // 16], i16, isOutput=False)
    padc_p = nc.declare_dram_parameter("padc", [128, LW], f32, isOutput=False)
    out_p = nc.declare_dram_parameter("out", [LW * 128, HF], f32, isOutput=True)

    table = nc.dram_tensor("table", [ROWS, TBL_ROW], bf16)

    Exp = mybir.ActivationFunctionType.Exp
    ADD = mybir.AluOpType.add
    SUB = mybir.AluOpType.subtract
    MULT = mybir.AluOpType.mult
    MAX = mybir.AluOpType.max

    with tile.TileContext(nc) as tc:
        with tc.tile_pool(name="const", bufs=1) as cpool:
            nc.gpsimd.load_library(library_config.mlp)

            # ---- constants ----
            wp_sb = cpool.tile([IN_DIM, HF], f32)
            nc.sync.dma_start(out=wp_sb[:], in_=wp_p[:, :])
            wn_sb = cpool.tile([IN_DIM, HF], f32)
            nc.sync.dma_start(out=wn_sb[:], in_=wn_p[:, :])
            att_sb = cpool.tile([128, 2 * HF], f32)
            nc.sync.dma_start(out=att_sb[:], in_=att_p[:, :])
            bias_sb = cpool.tile([128, HF], f32)
            nc.sync.dma_start(out=bias_sb[:], in_=bias_p[:, :])
            ident_f = cpool.tile([128, 128], f32)
            nc.sync.dma_start(out=ident_f[:], in_=ident_p[:, :])
            identb = cpool.tile([128, 128], bf16)
            nc.vector.tensor_copy(out=identb[:], in_=ident_f[:])
            padc_sb = cpool.tile([128, LW], f32)
            nc.sync.dma_start(out=padc_sb[:], in_=padc_p[:, :])
            c02b = cpool.tile([128, 4], bf16)
            nc.gpsimd.memset(c02b[:], NEG_SLOPE)

            # wext = [W_perm | as_vec | ad_vec] bf16 (264 cols)
            wext = cpool.tile([IN_DIM, TBL_USED], bf16)
            nc.vector.tensor_copy(out=wext[:, :HF], in_=wp_sb[:])
            # own-dst node rows [h|a_s|a_d] per window, SBUF-resident
            hown = cpool.tile([128, LW, TBL_USED], bf16)

            with (
                tc.tile_pool(name="ph1", bufs=2) as p1,
                tc.tile_pool(name="ph1ps", bufs=4, space="PSUM") as p1ps,
            ):
                prod = p1.tile([128, HF], f32, tag="prod")
                red = p1.tile([128, 8], f32, tag="red")
                nc.vector.tensor_tensor(
                    out=prod[:], in0=wn_sb[:], in1=att_sb[:, :HF], op=MULT
                )
                nc.vector.tensor_reduce(
                    out=red[:, 0:4],
                    in_=prod[:].rearrange("p (h f) -> p h f", h=HEADS),
                    axis=mybir.AxisListType.X,
                    op=ADD,
                )
                nc.vector.tensor_tensor(
                    out=prod[:], in0=wn_sb[:], in1=att_sb[:, HF:], op=MULT
                )
                nc.vector.tensor_reduce(
                    out=red[:, 4:8],
                    in_=prod[:].rearrange("p (h f) -> p h f", h=HEADS),
                    axis=mybir.AxisListType.X,
                    op=ADD,
                )
                nc.vector.tensor_copy(out=wext[:, HF:], in_=red[:])

                # ---- phase 1: node table ----
                for ci in range(N_XCH):
                    xc = p1.tile([IN_DIM, XCH], bf16, tag="xc")
                    nc.sync.dma_start(
                        out=xc[:], in_=xT[:, ci * XCH : (ci + 1) * XCH]
                    )
                    t1c = p1.tile([128, XCH // 128, TBL_USED], bf16, tag="t1c")
                    for t in range(XCH // 128):
                        hp = p1ps.tile([128, TBL_USED], f32, tag="hp")
                        nc.tensor.matmul(
                            out=hp[:],
                            lhsT=xc[:, t * 128 : (t + 1) * 128],
                            rhs=wext[:],
                            start=True,
                            stop=True,
                        )
                        # alternate PSUM drains between Scalar and Vector
                        if t % 2 == 0:
                            nc.scalar.copy(out=t1c[:, t, :], in_=hp[:])
                        else:
                            nc.vector.tensor_copy(out=t1c[:, t, :], in_=hp[:])
                    nc.sync.dma_start(
                        out=table[ci * XCH : (ci + 1) * XCH, :TBL_USED].rearrange(
                            "(t p) r -> p t r", p=128
                        ),
                        in_=t1c[:],
                    )
                # full node rows of own dst windows -> SBUF
                for i in range(LW):
                    xo = p1.tile([IN_DIM, 128], bf16, tag="xo")
                    nc.sync.dma_start(
                        out=xo[:], in_=xTo[:, i * 128 : (i + 1) * 128]
                    )
                    adp = p1ps.tile([128, TBL_USED], f32, tag="adp")
                    nc.tensor.matmul(
                        out=adp[:],
                        lhsT=xo[:],
                        rhs=wext[:],
                        start=True,
                        stop=True,
                    )
                    if i % 2 == 0:
                        nc.scalar.copy(out=hown[:, i, :], in_=adp[:])
                    else:
                        nc.vector.tensor_copy(out=hown[:, i, :], in_=adp[:])

            # ---- phase 2 ----
            gsems = (
                [nc.alloc_semaphore(f"gsem{k}") for k in range(8)]
                if GATHER_PREP
                else []
            )
            nsem = 0

            def gather(out_ap, in_ap, idxs_ap, n):
                nonlocal nsem
                if GATHER_PREP:
                    nc.gpsimd.dma_gather(
                        out_ap=out_ap, in_ap=in_ap, idxs_ap=idxs_ap,
                        num_idxs=n, num_idxs_reg=n, elem_size=TBL_ROW,
                        single_packet=SINGLE_PACKET, prepare_only=True,
                        sem=gsems[nsem % 8],
                    )
                    nsem += 1
                    nc.gpsimd.trigger_dma(count=None)
                else:
                    nc.gpsimd.dma_gather(
                        out_ap=out_ap, in_ap=in_ap, idxs_ap=idxs_ap,
                        num_idxs=n, num_idxs_reg=n, elem_size=TBL_ROW,
                        single_packet=SINGLE_PACKET,
                    )

            with (
                tc.tile_pool(name="gp", bufs=2) as gp,
                tc.tile_pool(name="sp", bufs=2) as sp,
                tc.tile_pool(name="ow", bufs=2) as owp,
                tc.tile_pool(name="pps", bufs=2, space="PSUM") as pps,
            ):
                Lrelu = mybir.ActivationFunctionType.Lrelu
                for (w0g, w1g, Ac, Bc, go) in groups:
                    T = Ac + Bc
                    nw = w1g - w0g
                    sd = sp.tile([128, T * 8], i16, tag="sd")
                    nc.sync.dma_start(
                        out=sd[:], in_=sd16_p[:, go // 16 : go // 16 + T * 8]
                    )
                    gt = gp.tile([128, T, TBL_ROW], bf16, tag="g")
                    if Ac:
                        gather(gt[:, :Ac, :], table[0:A_HI, :], sd[:, : Ac * 8],
                               Ac * 128)
                    if Bc:
                        gather(gt[:, Ac:, :], table[B_LO:ROWS, :], sd[:, Ac * 8 :],
                               Bc * 128)

                    # batched scores for the whole group:
                    # zg cols [0,T) edge z; [T, T+nw) self z; [T+nw, T+2nw) a_d
                    ZC = T + 2 * nw
                    zg = sp.tile([128, ZC, 4], bf16, tag="zg")
                    for j, w in enumerate(range(w0g, w1g)):
                        adww = hown[:, w, HF + 4 : HF + 8]
                        for (c0, cc) in (
                            (int(aoff[w]), int(lcapA[w])),
                            (int(boff[w]), int(lcapB[w])),
                        ):
                            if cc == 0:
                                continue
                            nc.vector.tensor_tensor(
                                out=zg[:, c0 : c0 + cc, :],
                                in0=gt[:, c0 : c0 + cc, HF : HF + 4],
                                in1=adww.rearrange(
                                    "p (o h) -> p o h", o=1
                                ).to_broadcast([128, cc, 4]),
                                op=ADD,
                            )
                        nc.vector.tensor_tensor(
                            out=zg[:, T + j, :], in0=hown[:, w, HF : HF + 4],
                            in1=adww, op=ADD,
                        )
                        nc.vector.tensor_copy(
                            out=zg[:, T + nw + j, :], in_=adww
                        )
                    # leaky relu: zg = max(zg, 0.2*zg) on DVE (batched)
                    zg2 = sp.tile([128, ZC, 4], bf16, tag="zg2")
                    nc.vector.tensor_tensor(
                        out=zg2[:],
                        in0=zg[:],
                        in1=c02b[:]
                        .rearrange("p (o h) -> p o h", o=1)
                        .to_broadcast([128, ZC, 4]),
                        op=MULT,
                    )
                    nc.vector.tensor_tensor(
                        out=zg[:], in0=zg[:], in1=zg2[:], op=MAX
                    )
                    nc.scalar.activation(
                        out=gt[:, :, HF + 4 : HF + 8], in_=zg[:, :T, :],
                        func=Exp,
                    )
                    ex2 = sp.tile([128, 2 * nw, 4], bf16, tag="ex2")
                    nc.scalar.activation(
                        out=ex2[:].rearrange("p c h -> p (c h)"),
                        in_=zg[:, T:, :].rearrange("p c h -> p (c h)"),
                        func=Exp,
                    )

                    for j, w in enumerate(range(w0g, w1g)):
                        pa = pps.tile([128, TBL_USED], f32, tag="pa")
                        slices = []
                        if lcapA[w]:
                            slices.append((int(aoff[w]), int(lcapA[w])))
                        if lcapB[w]:
                            slices.append((int(boff[w]), int(lcapB[w])))
                        nsl = 1 + sum(cc for _, cc in slices)
                        # self-loop message
                        exs = ex2[:, j, :]
                        ms = sp.tile([128, TBL_USED], bf16, tag="ms")
                        nc.vector.tensor_tensor(
                            out=ms[:, :HF].rearrange("p (f h) -> p f h", h=HEADS),
                            in0=hown[:, w, :HF].rearrange(
                                "p (f h) -> p f h", h=HEADS
                            ),
                            in1=exs.rearrange("p (o h) -> p o h", o=1)
                            .to_broadcast([128, OUT_DIM, HEADS]),
                            op=MULT,
                        )
                        nc.vector.tensor_copy(out=ms[:, HF : HF + 4], in_=exs)
                        nc.vector.tensor_copy(out=ms[:, HF + 4 : HF + 8], in_=exs)
                        nc.tensor.matmul(
                            out=pa[:], lhsT=identb[:], rhs=ms[:],
                            start=True, stop=(nsl == 1),
                        )
                        done = 1
                        for (c0, cc) in slices:
                            sl = gt[:, c0 : c0 + cc, :]
                            nc.vector.tensor_tensor(
                                out=sl[:, :, :HF].rearrange(
                                    "p c (f h) -> p c f h", h=HEADS
                                ),
                                in0=sl[:, :, :HF].rearrange(
                                    "p c (f h) -> p c f h", h=HEADS
                                ),
                                in1=sl[:, :, HF + 4 : HF + 8]
                                .rearrange("p c (o h) -> p c o h", o=1)
                                .to_broadcast([128, cc, OUT_DIM, HEADS]),
                                op=MULT,
                            )
                            for t in range(cc):
                                nc.tensor.matmul(
                                    out=pa[:],
                                    lhsT=identb[:],
                                    rhs=gt[:, c0 + t, :TBL_USED],
                                    start=False,
                                    stop=(done == nsl - 1),
                                )
                                done += 1

                        # finalize window
                        den = sp.tile([128, 4], f32, tag="den")
                        nc.vector.tensor_tensor(
                            out=den[:],
                            in0=padc_sb[:, w : w + 1].to_broadcast([128, 4]),
                            in1=ex2[:, nw + j, :],
                            op=MULT,
                        )
                        nc.vector.tensor_tensor(
                            out=den[:], in0=pa[:, HF + 4 : HF + 8], in1=den[:],
                            op=SUB,
                        )
                        rec = sp.tile([128, 4], f32, tag="rec")
                        nc.vector.reciprocal_approx_fast(out=rec[:], in_=den[:])
                        outw = owp.tile([128, HF], f32, tag="ow")
                        nc.vector.tensor_tensor(
                            out=outw[:].rearrange("p (h f) -> p h f", h=HEADS),
                            in0=pa[:, :HF].rearrange("p (f h) -> p h f", h=HEADS),
                            in1=rec[:]
                            .rearrange("p (h o) -> p h o", o=1)
                            .to_broadcast([128, HEADS, OUT_DIM]),
                            op=MULT,
                        )
                        nc.vector.tensor_tensor(
                            out=outw[:], in0=outw[:], in1=bias_sb[:], op=ADD
                        )
                        nc.sync.dma_start(
                            out=out_p[w * 128 : (w + 1) * 128, :], in_=outw[:]
                        )

    nc.compile()
    return nc


LAST_RES = None


def kernel(x, edge_index, W, att_src, att_dst, bias):
    x = np.asarray(x, dtype=np.float32)
    edge_index = np.asarray(edge_index)
    W = np.asarray(W, dtype=np.float32)
    att_src = np.asarray(att_src, dtype=np.float32)
    att_dst = np.asarray(att_dst, dtype=np.float32)
    bias = np.asarray(bias, dtype=np.float32)

    # self-loops handled by the kernel's direct path; only real edges here
    src = edge_index[0].astype(np.int64)
    dst = edge_index[1].astype(np.int64)

    (order, pos, sd16, padcnt, lcapA, lcapB, groups, aoff, boff, TOT) = (
        _prep_edges(src, dst)
    )

    # dense inputs (layout/dtype transforms only)
    xTf = np.zeros((IN_DIM, ROWS), dtype=np.float32)
    xTf[:, OFF : OFF + N_NODES] = x.T
    xT = xTf.astype(ml_dtypes.bfloat16)
    sorted_x = np.zeros((ROWS, IN_DIM), dtype=np.float32)
    sorted_x[pos] = x

    Wp = W.reshape(IN_DIM, HEADS, OUT_DIM).transpose(0, 2, 1).reshape(IN_DIM, HF)
    att_rep = np.zeros((128, 2 * HF), dtype=np.float32)
    att_rep[:, :HF] = att_src.reshape(1, HF)
    att_rep[:, HF:] = att_dst.reshape(1, HF)
    bias_rep = np.broadcast_to(bias.reshape(1, HF), (128, HF)).copy()
    ident = np.eye(128, dtype=np.float32)

    nc = _build_graph(lcapA, lcapB, groups, aoff, boff, TOT)

    in_maps = []
    for c in range(N_CORES):
        qidx = ((np.arange(LW)[:, None] * N_CORES + c) * 128
                + np.arange(128)[None, :]).ravel()
        xTo = sorted_x[qidx].T.astype(ml_dtypes.bfloat16).copy()
        in_maps.append(
            {
                "xT": xT, "xTo": xTo, "wp": Wp, "wn": W, "att": att_rep,
                "bias": bias_rep, "ident": ident, "sd16": sd16[c],
                "padc": padcnt[c],
            }
        )

    from concourse.bass_utils import run_bass_kernel_spmd

    res = run_bass_kernel_spmd(nc, in_maps, core_ids=list(range(N_CORES)))
    global LAST_RES
    LAST_RES = res
    outs = np.stack([res.results[c]["out"] for c in range(N_CORES)])
    full = outs.reshape(N_CORES, LW, 128, HF).transpose(1, 0, 2, 3).reshape(
        ROWS, HF
    )
    out = np.empty((N_NODES, HF), dtype=np.float32)
    out[order] = full[:N_NODES]
    return out
